# revision 1
# baseline (speedup 1.0000x reference)
"""CRF loss (BERT NER) Trainium2 kernel — v3.

result[b] = score[b] - log Z[b] for a 16-state linear-chain CRF,
S=512 steps, B=4096 sequences, data-parallel over 8 NeuronCores.

Host computes the tag-path score (cheap gathers) and the final
ln/sum of the per-segment dot products; the device computes the heavy
part of the normalizer log Z via a segment-probe factorization of the
linear-space forward recurrence  a_t = (E^T a_{t-1}) * g_t,
g_t = exp(e_t - C):

  Time is split into R=64 segments of L=8 steps. Each segment's transfer
  matrix M_r contracts the Hilbert projective metric by ~tanh(0.1)^L ~ 1e-8,
  i.e. it is rank-1 to far below the 2e-2 tolerance. We compute forward
  probes f_r = M_r 1 (segment 0 exactly from a_0), backward probes
  b_r = M_r^T 1 (last segment seeded with exp(end)), and combine:

    log Z = sum_m ln(b_{m+1}^T f_m) - sum_{m>=1} ln(1^T f_m) + S*C

  The device returns the raw dot products d_m = b_{m+1}^T f_m and
  denominators c_m = 1^T f_m (125 x 512 f32 per core); the host takes
  logs and sums (trivial).

  Probes for all segments advance IN PARALLEL (8 virtual steps).
  Partition packing: p = 8*j + c holds (state j, chunk c); column u of
  block m covers sequence b_local = 64*c + u of segment m. The per-step
  mix is a 128x128 block-diagonal matmul over 63 blocks x 64 columns.

g is stored PHASE-MAJOR: g[p, phase, seg, u] with t = 8*seg + phase, so
virtual step k of every segment reads one contiguous slab, and the DMA
stream (phases 0,7,1,6,2,5,3,4) unblocks both directions' step k after
~2 slabs — all 8 group pipelines start within ~4 us of kernel start.

Scheduling: raw Bass, static schedule, ONE counting semaphore per engine
(SP/PE/ACT/DVE/GPSIMD). Cross-engine dependencies are wait_ge on the
producer engine's cumulative instruction count (exact because engines
execute their programs in order). PSUM: one 512-col bank per
(direction, group); each virtual step flows through it in two
half-passes (matmul 512 -> consume 512). The PSUM consume+multiply work
is split between DVE (direct psum*g), ACT (evacuate) + DVE or GPSIMD
(multiply) per a static balance table.
"""

import numpy as np
import ml_dtypes

BF16 = ml_dtypes.bfloat16

S, B, T = 512, 4096, 16
NCORES = 8
BL = B // NCORES          # 512 sequences per core
NCH = 8                   # chunks per core (partition packing)
U = BL // NCH             # 64 columns per chunk
L = 8                     # segment length
R = S // L                # 64 segments
NF = R - 1                # 63 fwd blocks (= bwd blocks)
NOUT = NF + NF - 1        # 125 output rows (63 dots + 62 denominators)
C_SHIFT = 3.3             # per-step log-space recentering constant

FG = [(0, 16), (16, 32), (32, 48), (48, 63)]   # fwd block groups
BG = [(0, 15), (15, 31), (31, 47), (47, 63)]   # bwd block groups
PHASE_ORDER = [7, 0, 6, 1, 5, 2, 4, 3]         # DMA slab order

# consume-path per (dir, group, half): "d" = direct DVE psum*g,
# "a" = ACT copy + DVE mult, "g" = ACT copy + GPSIMD mult
FWD_PATH = {(0, 0): "d", (0, 1): "d", (1, 0): "d", (1, 1): "d",
            (2, 0): "d", (2, 1): "d", (3, 0): "a", (3, 1): "a"}
BWD_PATH = {(0, 0): "d", (0, 1): "a", (1, 0): "a", (1, 1): "g",
            (2, 0): "g", (2, 1): "g", (3, 0): "g", (3, 1): "g"}

_COMPILED = {}


def _build_bass():
    import concourse.bass as bass
    import concourse.mybir as mybir
    from contextlib import ExitStack

    f32 = mybir.dt.float32
    bf16 = mybir.dt.bfloat16
    Alu = mybir.AluOpType
    ActF = mybir.ActivationFunctionType

    nc = bass.Bass()

    g_in = nc.dram_tensor("g", [128, L, R, U], bf16, kind="ExternalInput")
    wcat_in = nc.dram_tensor("wcat", [128, 264], bf16, kind="ExternalInput")
    fc_in = nc.dram_tensor("fconst", [128, 3], f32, kind="ExternalInput")
    out_dram = nc.dram_tensor("dcout", [NCH, NOUT, U], f32, kind="ExternalOutput")

    with ExitStack() as ctx:
        g_sb = ctx.enter_context(nc.sbuf_tensor([128, L, R, U], bf16))
        wcat_sb = ctx.enter_context(nc.sbuf_tensor([128, 264], bf16))
        fc_sb = ctx.enter_context(nc.sbuf_tensor([128, 3], f32))
        F_sb = ctx.enter_context(nc.sbuf_tensor([128, NF, U], bf16))
        B_sb = ctx.enter_context(nc.sbuf_tensor([128, NF, U], bf16))
        P_sb = ctx.enter_context(nc.sbuf_tensor([128, NF, U], bf16))
        Etmp = [ctx.enter_context(nc.sbuf_tensor(f"etmp{gi}", [128, 1024], bf16))
                for gi in range(4)]      # fwd evacuation buffers
        H_sb = [ctx.enter_context(nc.sbuf_tensor(f"hbuf{gi}", [128, 1024], bf16))
                for gi in range(4)]      # bwd evacuation buffers
        lnout_sb = ctx.enter_context(nc.sbuf_tensor([NCH, NOUT * U], f32))
        qf = [ctx.enter_context(nc.psum_tensor(f"qf{i}", [128, 512], f32))
              for i in range(4)]
        qb = [ctx.enter_context(nc.psum_tensor(f"qb{i}", [128, 512], f32))
              for i in range(4)]

        sems = {e: ctx.enter_context(nc.semaphore(f"s_{e}"))
                for e in ("sp", "pe", "act", "dve", "gp", "gpdma")}
        block = ctx.enter_context(nc.Block())

        Fflat = F_sb[:].rearrange("p r u -> p (r u)")
        Bflat = B_sb[:].rearrange("p r u -> p (r u)")
        Pflat = P_sb[:].rearrange("p r u -> p (r u)")
        WE = wcat_sb[:, 0:128]
        WET = wcat_sb[:, 128:256]
        W1 = wcat_sb[:, 256:264]
        SC = fc_sb[:, 0:1]
        ZC = fc_sb[:, 1:2]
        CS = fc_sb[:, 2:3]

        # ---------------- static schedule construction ----------------
        PROG = {e: [] for e in sems}
        cnt = {e: 0 for e in sems}

        def emit(eng, fn, waits=(), inc=1, inc_sem=None):
            sem = inc_sem or eng
            PROG[eng].append((fn, [w for w in waits if w is not None], inc, sem))
            cnt[sem] += inc
            return (sem, cnt[sem])

        mk_wcat = ("sp", 16)
        mk_fc = ("sp", 32)
        # g streams over TWO DMA queues (real HW: independent DGE rings;
        # the cost model serializes them, so this is sim-neutral):
        # GPSIMD-issued queue carries phases 0..3 (forward's early steps),
        # SP carries 7..4 (backward's early steps) after the constants.
        # Each queue has its own counting semaphore; each phase slab lands
        # in two halves so low-seg groups unblock half a slab earlier.
        SP_PHASES = [7, 6, 5, 4, 3]
        GP_PHASES = [0, 1, 2]
        mk_phase_lo, mk_phase = {}, {}
        emit("sp", lambda q: q.dma_start(wcat_sb[:], wcat_in[:]), inc=16)
        emit("sp", lambda q: q.dma_start(fc_sb[:], fc_in[:]), inc=16)
        for i, ph in enumerate(SP_PHASES):
            mk_phase_lo[ph] = ("sp", 48 + 32 * i)
            mk_phase[ph] = ("sp", 64 + 32 * i)
        for i, ph in enumerate(GP_PHASES):
            mk_phase_lo[ph] = ("gpdma", 16 + 32 * i)
            mk_phase[ph] = ("gpdma", 32 + 32 * i)
        for ph in SP_PHASES:
            emit("sp", lambda q, ph=ph: q.dma_start(
                g_sb[:, ph, 0:32, :], g_in[:, ph, 0:32, :]), inc=16)
            emit("sp", lambda q, ph=ph: q.dma_start(
                g_sb[:, ph, 32:64, :], g_in[:, ph, 32:64, :]), inc=16)

        for ph in GP_PHASES:
            emit("gp", lambda q, ph=ph: q.dma_start(
                g_sb[:, ph, 0:32, :], g_in[:, ph, 0:32, :]),
                inc=16, inc_sem="gpdma")
            emit("gp", lambda q, ph=ph: q.dma_start(
                g_sb[:, ph, 32:64, :], g_in[:, ph, 32:64, :]),
                inc=16, inc_sem="gpdma")

        def mk_ph(ph, gi):
            return mk_phase_lo[ph] if gi < 2 else mk_phase[ph]

        f_ready = [None] * 4   # F complete for last vstep (dve)
        f_free = [None] * 4    # fwd psum bank free
        f_hfree = [[None, None] for _ in range(4)]   # Etmp half free
        b_ready = [None] * 4
        b_free = [None] * 4
        b_hfree = [[None, None] for _ in range(4)]
        pdots = [None] * 4

        def halves(lo, hi):
            out = []
            b0 = lo
            while b0 < hi:
                b1 = min(hi, b0 + 8)
                out.append((b0, b1))
                b0 = b1
            return out

        def consume(path, eng_buf, psum, ncols, h, out_ap, gsl, waits,
                    hfree, mult_extra_wait):
            """evacuate+multiply one half-pass; returns (state_mk, psum_free_mk)."""
            if path == "d":
                mk = emit("dve", lambda q: nc.vector.tensor_tensor(
                    out=out_ap, in0=psum[:, 0:ncols], in1=gsl, op=Alu.mult),
                    waits + [mult_extra_wait])
                return mk, mk, None
            mk_cp = emit("act", lambda q: nc.scalar.copy(
                eng_buf[:, h * 512: h * 512 + ncols], psum[:, 0:ncols]),
                waits + ([hfree[h]] if hfree[h] else []))
            meng = "dve" if path == "a" else "gp"
            mk = emit(meng, lambda q: getattr(
                nc, "vector" if meng == "dve" else "gpsimd").tensor_tensor(
                out=out_ap, in0=eng_buf[:, h * 512: h * 512 + ncols], in1=gsl,
                op=Alu.mult), [mk_cp, mult_extra_wait])
            hfree[h] = mk
            return mk, mk_cp, mk

        def fwd_unit(gi, k, h, blo, bhi):
            ncols = (bhi - blo) * U
            waits = [mk_wcat, f_ready[gi]]
            if f_free[gi] and f_free[gi] != f_ready[gi]:
                waits.append(f_free[gi])
            mk_mm = emit("pe", lambda q: nc.tensor.matmul(
                qf[gi][:, 0:ncols], WE, Fflat[:, blo * U:bhi * U],
                start=True, stop=True), waits)
            gsl = g_sb[:, k, blo:bhi, :]
            mk, free_mk, _ = consume(
                FWD_PATH[(gi, h)], Etmp[gi], qf[gi], ncols, h,
                F_sb[:, blo:bhi, :], gsl, [mk_mm], f_hfree[gi], mk_ph(k, gi))
            f_free[gi] = free_mk
            return mk

        def bwd_unit(gi, k, h, blo, bhi):
            ncols = (bhi - blo) * U
            waits = [mk_wcat, b_ready[gi]]
            if b_free[gi] and b_free[gi] != b_ready[gi]:
                waits.append(b_free[gi])
            mk_mm = emit("pe", lambda q: nc.tensor.matmul(
                qb[gi][:, 0:ncols], WET, Bflat[:, blo * U:bhi * U],
                start=True, stop=True), waits)
            gsl = g_sb[:, 7 - k, blo + 1:bhi + 1, :]
            mk, free_mk, _ = consume(
                BWD_PATH[(gi, h)], H_sb[gi], qb[gi], ncols, h,
                B_sb[:, blo:bhi, :], gsl, [mk_mm], b_hfree[gi], mk_ph(7 - k, gi))
            b_free[gi] = free_mk
            return mk

        def bwd_final_unit(gi, h, blo, bhi):
            ncols = (bhi - blo) * U
            waits = [mk_wcat, b_ready[gi]]
            if b_free[gi] and b_free[gi] != b_ready[gi]:
                waits.append(b_free[gi])
            mk_mm = emit("pe", lambda q: nc.tensor.matmul(
                qb[gi][:, 0:ncols], WET, Bflat[:, blo * U:bhi * U],
                start=True, stop=True), waits)
            mk = emit("dve", lambda q: nc.vector.tensor_tensor(
                out=P_sb[:, blo:bhi, :], in0=qb[gi][:, 0:ncols],
                in1=F_sb[:, blo:bhi, :], op=Alu.mult), [mk_mm])
            b_free[gi] = mk
            return mk

        def tc_half(gi, h, prev_ln):
            """one half of c_m = colsum(F) through the freed qf bank."""
            lo, hi = BG[gi]
            clo = max(lo, 1)
            b0, b1 = halves(clo, hi)[h]
            ncols = (b1 - b0) * U
            waits = [f_ready[gi], prev_ln]
            if h == 0 and f_free[gi] and f_free[gi][0] == "act":
                waits.append(f_free[gi])
            mm = emit("pe", lambda q, b0=b0, b1=b1, ncols=ncols:
                      nc.tensor.matmul(qf[gi][0:NCH, 0:ncols], W1,
                                       Fflat[:, b0 * U:b1 * U],
                                       start=True, stop=True), waits)
            return emit("act", lambda q, b0=b0, ncols=ncols:
                        nc.scalar.activation(
                            lnout_sb[:, (NF + b0 - 1) * U:
                                     (NF + b0 - 1) * U + ncols],
                            qf[gi][0:NCH, 0:ncols], ActF.Ln), [mm])

        def td_half(gi, h, p_mk, ln_c_mk):
            """one half of d_m = colsum(P); h0 via qf (after Ln-c), h1 via qb."""
            lo, hi = BG[gi]
            b0, b1 = halves(lo, hi)[h]
            ncols = (b1 - b0) * U
            ps = qf[gi] if h == 0 else qb[gi]
            mm = emit("pe", lambda q, ps=ps, b0=b0, b1=b1, ncols=ncols:
                      nc.tensor.matmul(ps[0:NCH, 0:ncols], W1,
                                       Pflat[:, b0 * U:b1 * U],
                                       start=True, stop=True),
                      [p_mk, ln_c_mk if h == 0 else None])
            if gi >= 2:
                # DVE is idle after the P dots while ACT drains its Ln queue:
                # evacuate these groups raw on DVE; the host logs rows 31:63
                return emit("dve", lambda q, ps=ps, b0=b0, ncols=ncols:
                            nc.vector.tensor_copy(
                                lnout_sb[:, b0 * U:b0 * U + ncols],
                                ps[0:NCH, 0:ncols]), [mm])
            return emit("act", lambda q, ps=ps, b0=b0, ncols=ncols:
                        nc.scalar.activation(
                            lnout_sb[:, b0 * U:b0 * U + ncols],
                            ps[0:NCH, 0:ncols], ActF.Ln), [mm])

        # ---------------- wave loop (all groups start at wave 0) -------
        for w in range(10):
            if w == 0:
                for gi in range(4):
                    flo, fhi = FG[gi]
                    if gi == 0:
                        emit("dve", lambda q: nc.vector.tensor_scalar(
                            out=F_sb[:, 0, :], in0=g_sb[:, 0, 0, :],
                            scalar1=SC, scalar2=None, op0=Alu.mult),
                            [mk_phase_lo[0], mk_fc])
                        f_ready[0] = emit("dve", lambda q: nc.vector.tensor_scalar(
                            out=F_sb[:, 1:16, :], in0=g_sb[:, 0, 1:16, :],
                            scalar1=CS, scalar2=None, op0=Alu.mult), [])
                    else:
                        f_ready[gi] = emit(
                            "dve", lambda q, flo=flo, fhi=fhi:
                            nc.vector.tensor_scalar(
                                out=F_sb[:, flo:fhi, :],
                                in0=g_sb[:, 0, flo:fhi, :],
                                scalar1=CS, scalar2=None, op0=Alu.mult),
                            [mk_ph(0, gi), mk_fc])
                    blo, bhi = BG[gi]
                    if gi < 3:
                        b_ready[gi] = emit(
                            "gp", lambda q, blo=blo, bhi=bhi:
                            nc.gpsimd.tensor_copy(
                                B_sb[:, blo:bhi, :],
                                g_sb[:, 7, blo + 1:bhi + 1, :]),
                            [mk_ph(7, gi)])
                    else:
                        emit("gp", lambda q, blo=blo, bhi=bhi:
                             nc.gpsimd.tensor_copy(
                                 B_sb[:, blo:bhi, :],
                                 g_sb[:, 7, blo + 1:bhi + 1, :]),
                             [mk_phase[7], mk_fc])
                        b_ready[3] = emit("gp", lambda q: nc.gpsimd.tensor_scalar(
                            out=B_sb[:, NF - 1, :], in0=B_sb[:, NF - 1, :],
                            scalar1=ZC, scalar2=None, op0=Alu.mult), [])
                continue
            k = w
            if 1 <= k <= 7:
                for h in range(2):
                    for gi in range(4):
                        fh = halves(*FG[gi])
                        mk = fwd_unit(gi, k, h, *fh[h])
                        if h == 1:
                            f_ready[gi] = mk
                        bh = halves(*BG[gi])
                        mk = bwd_unit(gi, k, h, *bh[h])
                        if h == 1:
                            b_ready[gi] = mk
            elif k == 8:
                # interleave: c-sum halves keep PE busy between the bare-E
                # finals; d-sums chase each group's P dot as it lands.
                ln_c0 = [tc_half(gi, 0, None) for gi in range(4)]
                pt0 = [bwd_final_unit(gi, 0, *halves(*BG[gi])[0])
                       for gi in range(4)]
                ln_c1 = [tc_half(gi, 1, ln_c0[gi]) for gi in range(4)]
                pt1 = [bwd_final_unit(gi, 1, *halves(*BG[gi])[1])
                       for gi in range(4)]
                for gi in range(4):
                    lo, hi = BG[gi]
                    clo = max(lo, 1)
                    emit("sp", lambda q, gi=gi, clo=clo, hi=hi: q.dma_start(
                        out_dram[:, NF + clo - 1:NF + hi - 1, :],
                        lnout_sb[:, (NF + clo - 1) * U:(NF + hi - 1) * U]
                        .rearrange("p (r u) -> p r u", u=U)),
                        [ln_c1[gi]], inc=16)
                ln_d1 = [None] * 4
                for gi in range(4):
                    td_half(gi, 0, pt0[gi], ln_c1[gi])
                for gi in range(4):
                    ln_d1[gi] = td_half(gi, 1, pt1[gi], None)
                for gi in range(4):
                    lo, hi = BG[gi]
                    emit("sp", lambda q, gi=gi, lo=lo, hi=hi: q.dma_start(
                        out_dram[:, lo:hi, :],
                        lnout_sb[:, lo * U:hi * U]
                        .rearrange("p (r u) -> p r u", u=U)),
                        [ln_d1[gi]], inc=16)

        # ---------------- emission ----------------
        def run(eng, q):
            hwm = {}
            for fn, waits, inc, inc_sem in PROG[eng]:
                best = {}
                for (weng, wcnt) in waits:
                    if weng == eng:
                        continue
                    best[weng] = max(best.get(weng, 0), wcnt)
                for weng, wcnt in best.items():
                    if hwm.get(weng, 0) < wcnt:
                        q.wait_ge(sems[weng], wcnt)
                        hwm[weng] = wcnt
                instr = fn(q)
                instr.then_inc(sems[inc_sem], inc)

        @block.sync
        def _(sync):
            run("sp", sync)

        @block.tensor
        def _(tensor):
            run("pe", tensor)

        @block.scalar
        def _(scalar):
            run("act", scalar)

        @block.vector
        def _(vector):
            run("dve", vector)

        @block.gpsimd
        def _(gp):
            run("gp", gp)

    return nc


def _prep_core_inputs(emissions, start_transitions, end_transitions, transitions):
    """Host-side reshaping: returns per-core input dicts."""
    E = np.exp(transitions.astype(np.float64)).astype(np.float32)
    W = np.zeros((128, 128), np.float32)
    for c in range(NCH):
        W[c::NCH, c::NCH] = E
    W1 = np.zeros((128, NCH), np.float32)
    for c in range(NCH):
        W1[c::NCH, c] = 1.0
    wcat = np.concatenate([W, W.T, W1], axis=1).astype(BF16)  # [128, 264]

    j_of_p = np.arange(128) // NCH
    cs128 = E.astype(BF16).astype(np.float32).sum(axis=0)[j_of_p]
    fconst = np.stack([
        np.exp(start_transitions.astype(np.float64))[j_of_p].astype(np.float32),
        np.exp(end_transitions.astype(np.float64))[j_of_p].astype(np.float32),
        cs128.astype(np.float32),
    ], axis=1)  # [128, 3]

    # g[core, p=8j+c, phase, seg, u] = exp(e[8*seg+phase, 512*core+64*c+u, j] - C)
    e6 = emissions.reshape(R, L, NCORES, NCH, U, T)   # [seg, ph, core, c, u, j]
    g32 = np.exp(e6 - np.float32(C_SHIFT))
    gb = g32.astype(BF16)
    g = np.ascontiguousarray(gb.transpose(2, 5, 3, 1, 0, 4))  # [core, j, c, ph, seg, u]
    g = g.reshape(NCORES, 128, L, R, U)

    return [
        {"g": g[core], "wcat": wcat, "fconst": fconst}
        for core in range(NCORES)
    ]


def _host_score(emissions, tags, masks, start_transitions, end_transitions,
                transitions):
    tags = tags.astype(np.int64)
    b_idx = np.arange(B)
    score = start_transitions[tags[0]] + emissions[0, b_idx, tags[0]]
    trans_sc = transitions[tags[:-1], tags[1:]] * masks[1:]
    emit_sc = np.take_along_axis(
        emissions[1:], tags[1:, :, None], axis=2)[:, :, 0] * masks[1:]
    score = score + trans_sc.sum(0) + emit_sc.sum(0)
    seq_ends = masks.astype(np.int32).sum(0) - 1
    last_tags = tags[seq_ends, b_idx]
    return score + end_transitions[last_tags]


def _host_normalizer(emissions, masks, start_transitions, end_transitions,
                     transitions):
    """Full-precision host fallback (only used when masks aren't all ones)."""
    sc = (start_transitions[None] + emissions[0]).astype(np.float64)
    E64 = np.exp(transitions.astype(np.float64))
    for t in range(1, S):
        m = sc.max(1, keepdims=True)
        nxt = m + np.log(np.exp(sc - m) @ E64) + emissions[t]
        keep = masks[t][:, None] > 0
        sc = np.where(keep, nxt, sc)
    m = sc.max(1, keepdims=True)
    return (
        m[:, 0]
        + np.log(np.exp(sc - m + end_transitions[None]).sum(1))
    ).astype(np.float32)


def kernel(emissions, tags, masks, start_transitions, end_transitions,
           transitions):
    emissions = np.asarray(emissions, np.float32)
    masks_np = np.asarray(masks, np.float32)
    tags_np = np.asarray(tags)
    start_np = np.asarray(start_transitions, np.float32)
    end_np = np.asarray(end_transitions, np.float32)
    trans_np = np.asarray(transitions, np.float32)

    score = _host_score(emissions, tags_np, masks_np, start_np, end_np,
                        trans_np)

    if not np.all(masks_np == 1.0):
        norm = _host_normalizer(emissions, masks_np, start_np, end_np,
                                trans_np)
        return (score - norm).astype(np.float32)

    from concourse.bass_utils import run_bass_kernel_spmd

    if "nc" not in _COMPILED:
        _COMPILED["nc"] = _build_bass()
    nc = _COMPILED["nc"]

    in_maps = _prep_core_inputs(emissions, start_np, end_np, trans_np)
    res = run_bass_kernel_spmd(nc, in_maps, core_ids=list(range(NCORES)))

    norm = np.empty((NCORES, BL), np.float32)
    for core in range(NCORES):
        dc = res.results[core]["dcout"].astype(np.float64)  # [NCH, NOUT, U]
        dc[:, 31:NF, :] = np.log(dc[:, 31:NF, :])  # groups 2,3 arrive raw
        norm[core] = (
            dc[:, 0:NF, :].sum(axis=1) - dc[:, NF:NOUT, :].sum(axis=1)
        ).astype(np.float32).reshape(BL)
    norm = norm.reshape(B) + np.float32(S * C_SHIFT)
    return (score - norm).astype(np.float32)



# revision 7
# speedup vs baseline: 1.4121x; 1.4121x over previous
"""CRF loss (BERT NER) Trainium2 kernel — v5.

result[b] = score[b] - log Z[b] for a 16-state linear-chain CRF,
S=512 steps, B=4096 sequences, data-parallel over 8 NeuronCores.
Host computes the tag-path score (cheap gathers); the device computes the
heavy part of the normalizer log Z.

Algorithm: truncated-left-probe telescoping of the linear-space forward
recurrence  a_t = (E^T a_{t-1}) * g_t,  g_t = exp(e_t - C).

  Time is split into R=64 segments of L=8 steps.  Each segment's transfer
  matrix M_m = D_7 E^T ... D_0 E^T contracts the Hilbert projective metric
  by ~tanh(0.1)^8, i.e. it is rank-1 far below the 2e-2 tolerance.  With
  forward probes f_m = M_m 1 (f_0 = M_0 a_0 exactly) and rank-1
  M_m ~= f_m q_m^T / (q_m^T 1) for ANY probe q_m not orthogonal to the
  left factor, the chain telescopes to

    log Z = ln(u^T f_{R-1})
          + sum_{m=1}^{R-1} [ ln(q_m^T f_{m-1}) - ln(q_m^T 1) ]  + S*C .

  The left probe is truncated to ONE factor: q_m = E g_{m,7} — a single
  matmul straight from the g slab (numerically validated: max |logZ err|
  ~0.2 in bf16, ~1.2 with fp8 g, vs an absolute budget of ~33).
  q_m^T 1 = cs_E . g_{m,7} is a pure function of the inputs -> host.
  Segment 0's exp(start)/cs seed is folded into its ph0 g data (rescaled
  by alpha for fp8 range; ln(alpha) subtracted on the host).

  Device work per core (512 sequences = 8 chunks x 64 cols, 64 segments):
  7 recurrence waves of (block-diag 128x128 matmul + elementwise *g) with
  the init folded into the first matmul's weights, then q matmuls,
  P = q * f_{m-1}, and 17 packed dot matmuls landing every q_m^T f_{m-1}
  (and u^T f_{R-1}) in one [128, 256] f32 psum tile -> one small DMA out.
  Host takes the logs and telescopes.

  Structure: 8 independent half-chains of 8 segments (512 cols), one PSUM
  bank each, so engines stay fed and chains can skew across waves.  Each
  (wave, half-chain) cell is statically assigned a consume path,
    'd' — DVE  mult directly from PSUM        (~658 ns / 512 cols)
    'a' — ACT  copy->SBUF bf16 + DVE 2x mult  (612 + 326 ns)
    'g' — ACT  copy->SBUF bf16 + GpSimd mult  (612 + 1111 ns)
  per the PATH table (rotated; rows mix 3d2a3g / 4d2a2g to balance DVE /
  ACT / GpSimd).  'a' cells need bf16 g slabs (DVE 2x needs 2-byte
  operands); all other slabs stream as fp8e4m3 to cut DMA.

Scheduling: raw Bass, static schedule, one counting semaphore per engine;
cross-engine deps are wait_ge on the producer engine's cumulative count.
DMA completions are OUT OF ORDER across hardware queues, so every DMA
wait targets a dedicated semaphore (per-phase; split g8/g16 sems for the
startup-critical phases 1-2, shared >=32 waits later).
"""

import numpy as np
import ml_dtypes

BF16 = ml_dtypes.bfloat16
FP8 = ml_dtypes.float8_e4m3fn

S, B, T = 512, 4096, 16
NCORES = 8
BL = B // NCORES          # 512 sequences per core
NCH = 8                   # chunks per core (partition packing p = 8*j + c)
U = 64                    # columns per (segment, chunk)
L = 8                     # segment length
R = S // L                # 64 segments
NHC = 8                   # half-chains of 8 segments (512 cols each)
SEGH = R // NHC
C_SHIFT = 3.3             # per-step log-space recentering constant


def _rot(s, k):
    return s[k % len(s):] + s[:k % len(s)]


_PAT1 = "dagdgagd"        # 3d 2a 3g
_PAT2 = "dgaddgad"        # 4d 2a 2g
# consume path per (vstep 1..7, half-chain); P row for the q*f pass
PATH = [_rot(_PAT1, 0), _rot(_PAT2, 1), _rot(_PAT1, 2), _rot(_PAT2, 3),
        _rot(_PAT1, 4), _rot(_PAT2, 5), _rot(_PAT1, 6)]
PPATH = _rot(_PAT1, 3)


def _regions():
    """g slab regions: (phase, hc) -> index into the fp8 / bf16 tensor."""
    reg8, reg16 = {}, {}
    for h in range(NHC):
        reg8[(0, h)] = len(reg8)
    for ph in range(1, L):
        for h in range(NHC):
            if PATH[ph - 1][h] != "a":
                reg8[(ph, h)] = len(reg8)
    for ph in range(1, L):
        for h in range(NHC):
            if PATH[ph - 1][h] == "a":
                reg16[(ph, h)] = len(reg16)
    return reg8, reg16


REG8, REG16 = _regions()
NR8, NR16 = len(REG8), len(REG16)

_COMPILED = {}


def _build_bass():
    import concourse.bass as bass
    import concourse.mybir as mybir
    from contextlib import ExitStack

    f32 = mybir.dt.float32
    bf16 = mybir.dt.bfloat16
    fp8 = mybir.dt.float8e4
    Alu = mybir.AluOpType

    nc = bass.Bass()
    g8_in = nc.dram_tensor("g8", [128, NR8, 512], fp8, kind="ExternalInput")
    g16_in = nc.dram_tensor("g16", [128, NR16, 512], bf16, kind="ExternalInput")
    w1_in = nc.dram_tensor("w1", [128, 256], bf16, kind="ExternalInput")
    w2_in = nc.dram_tensor("w2", [128, 2304], bf16, kind="ExternalInput")
    dout = nc.dram_tensor("dout", [128, 256], f32, kind="ExternalOutput")

    with ExitStack() as ctx:
        g8_sb = ctx.enter_context(nc.sbuf_tensor([128, NR8, 512], fp8))
        g16_sb = ctx.enter_context(nc.sbuf_tensor([128, NR16, 512], bf16))
        w1_sb = ctx.enter_context(nc.sbuf_tensor([128, 256], bf16))
        w2_sb = ctx.enter_context(nc.sbuf_tensor([128, 2304], bf16))
        F_sb = ctx.enter_context(nc.sbuf_tensor([128, R * U], bf16))
        P_sb = ctx.enter_context(nc.sbuf_tensor([128, R - 1, U], bf16))
        EV = [ctx.enter_context(nc.sbuf_tensor(f"ev{h}", [128, 512], bf16))
              for h in range(NHC)]
        dsb = ctx.enter_context(nc.sbuf_tensor([128, 256], f32))
        PS = [ctx.enter_context(nc.psum_tensor(f"ps{h}", [128, 512], f32))
              for h in range(NHC)]
        semnames = (["sp", "pe", "act", "dve", "gp", "w1", "w2", "p0a", "p0b",
                     "p1a", "p1b", "p2a", "p2b"]
                    + [f"p{ph}" for ph in range(3, L)])
        sems = {e: ctx.enter_context(nc.semaphore(f"s_{e}"))
                for e in semnames}
        block = ctx.enter_context(nc.Block())

        WE = w1_sb[:, 0:128]
        W0cs = w1_sb[:, 128:256]
        WQ = w2_sb[:, 0:128]
        Wd = [w2_sb[:, 128 + 128 * r:256 + 128 * r] for r in range(16)]
        Wu = w2_sb[:, 2176:2304]
        Pf = P_sb[:].rearrange("p s u -> p (s u)")

        # ---------------- static schedule construction ----------------
        PROG = {e: [] for e in ("sp", "pe", "act", "dve", "gp")}
        cnt = {e: 0 for e in sems}

        def emit(eng, fn, waits=(), inc=1, sem=None):
            sem = sem or eng
            PROG[eng].append((fn, [w for w in waits if w is not None], inc,
                              sem))
            cnt[sem] += inc
            return (sem, cnt[sem])

        def dma(dst, src):
            return lambda q: q.dma_start(dst, src)

        # ---- DMA stream (sp). Completions are OUT OF ORDER across hw
        # queues: every wait targets a dedicated sem. Phases 1-2 (startup
        # critical) get split g8/g16 sems; later phases share one sem and
        # consumers wait for both transfers (>=32).
        mk_w1 = emit("sp", dma(w1_sb[:], w1_in[:]), inc=16, sem="w1")
        g8mk, g16mk = {}, {}
        mk = emit("sp", dma(g8_sb[:, 0:4, :], g8_in[:, 0:4, :]), inc=16,
                  sem="p0a")
        for h in range(4):
            g8mk[(0, h)] = mk
        mk = emit("sp", dma(g8_sb[:, 4:8, :], g8_in[:, 4:8, :]), inc=16,
                  sem="p0b")
        for h in range(4, 8):
            g8mk[(0, h)] = mk
        mk_w2 = None
        for ph in range(1, L):
            idx8 = [REG8[(ph, h)] for h in range(NHC) if (ph, h) in REG8]
            lo, hi = min(idx8), max(idx8) + 1
            assert hi - lo == len(idx8)
            idx16 = [REG16[(ph, h)] for h in range(NHC) if (ph, h) in REG16]
            lo6, hi6 = min(idx16), max(idx16) + 1
            assert hi6 - lo6 == len(idx16)
            if ph <= 2:
                s8, s16 = f"p{ph}a", f"p{ph}b"
            else:
                s8 = s16 = f"p{ph}"
            mk8 = emit("sp", dma(g8_sb[:, lo:hi, :], g8_in[:, lo:hi, :]),
                       inc=16, sem=s8)
            mk16 = emit("sp", dma(g16_sb[:, lo6:hi6, :],
                                  g16_in[:, lo6:hi6, :]), inc=16, sem=s16)
            if s8 == s16:
                mk8 = mk16 = (s8, cnt[s8])   # shared sem: wait both (>=32)
            for h in range(NHC):
                if (ph, h) in REG8:
                    g8mk[(ph, h)] = mk8
                if (ph, h) in REG16:
                    g16mk[(ph, h)] = mk16
            if ph == 3:
                mk_w2 = emit("sp", dma(w2_sb[:], w2_in[:]), inc=16, sem="w2")

        def slab(ph, h):
            if (ph, h) in REG8:
                return g8_sb[:, REG8[(ph, h)], :], g8mk[(ph, h)]
            return g16_sb[:, REG16[(ph, h)], :], g16mk[(ph, h)]

        consume_mk = [None] * NHC
        ev_mk = [None] * NHC

        def consume_wave(k, mm_mk):
            """Emit one wave of consumes: F[:, hc] = PS[hc] * g(phase k).
            ACT copies for 'g' cells go first so GpSimd starts earliest."""
            paths = PATH[k - 1]
            cp_mk = {}
            for h in [h for h in range(NHC) if paths[h] == "g"] + \
                     [h for h in range(NHC) if paths[h] == "a"]:
                cp_mk[h] = emit("act", lambda q, h=h: nc.scalar.copy(
                    EV[h][:], PS[h][:]), [mm_mk[h], ev_mk[h]])
            for h in range(NHC):
                gsl, gmk = slab(k, h)
                outF = F_sb[:, 512 * h:512 * h + 512]
                if paths[h] == "d":
                    mk = emit("dve", lambda q, h=h, gsl=gsl, outF=outF:
                              nc.vector.tensor_tensor(
                                  out=outF, in0=PS[h][:], in1=gsl,
                                  op=Alu.mult), [mm_mk[h], gmk])
                else:
                    eng = "dve" if paths[h] == "a" else "gp"
                    obj = nc.vector if paths[h] == "a" else nc.gpsimd
                    mk = emit(eng, lambda q, h=h, gsl=gsl, outF=outF, obj=obj:
                              obj.tensor_tensor(
                                  out=outF, in0=EV[h][:], in1=gsl,
                                  op=Alu.mult), [cp_mk[h], gmk])
                    ev_mk[h] = mk
                consume_mk[h] = mk

        # ---- vstep 1: init matmuls read ph0 slabs with folded weights ----
        mm_mk = [None] * NHC
        for h in range(NHC):
            s0, smk = slab(0, h)
            mm_mk[h] = emit("pe", lambda q, h=h, s0=s0: nc.tensor.matmul(
                PS[h][:], W0cs, s0[:], start=True, stop=True), [mk_w1, smk])
        consume_wave(1, mm_mk)

        # ---- vsteps 2..7: recurrence matmuls on F ----
        for k in range(2, L):
            for h in range(NHC):
                mm_mk[h] = emit("pe", lambda q, h=h: nc.tensor.matmul(
                    PS[h][:], WE, F_sb[:, 512 * h:512 * h + 512],
                    start=True, stop=True), [consume_mk[h]])
            consume_wave(k, mm_mk)

        # ---- q matmuls (straight off the ph7 slabs) + P = q * f_{m-1} ----
        for h in range(NHC):
            gsl, gmk = slab(L - 1, h)
            lo = 64 if h == 0 else 0   # no q for segment 0
            mm_mk[h] = emit("pe", lambda q, h=h, gsl=gsl, lo=lo:
                            nc.tensor.matmul(
                                PS[h][:, lo:512], WQ, gsl[:, lo:512],
                                start=True, stop=True),
                            [consume_mk[h], mk_w2, gmk])

        p_mk = [None] * NHC
        cp_mk = {}
        for h in [h for h in range(NHC) if PPATH[h] == "g"] + \
                 [h for h in range(NHC) if PPATH[h] == "a"]:
            lo = 64 if h == 0 else 0
            waits = [mm_mk[h], consume_mk[h], ev_mk[h]]
            if h > 0:
                waits.append(consume_mk[h - 1])
            cp_mk[h] = emit("act", lambda q, h=h, lo=lo: nc.scalar.copy(
                EV[h][:, lo:512], PS[h][:, lo:512]), waits)
        for h in range(NHC):
            lo = 64 if h == 0 else 0
            fin = F_sb[:, 512 * h + lo - 64:512 * h + 448]
            outP = Pf[:, 512 * h + lo - 64:512 * h + 448]
            if PPATH[h] == "d":
                waits = [mm_mk[h], consume_mk[h]]
                if h > 0:
                    waits.append(consume_mk[h - 1])
                p_mk[h] = emit("dve", lambda q, h=h, lo=lo, fin=fin,
                               outP=outP: nc.vector.tensor_tensor(
                                   out=outP, in0=PS[h][:, lo:512], in1=fin,
                                   op=Alu.mult), waits)
            else:
                eng = "dve" if PPATH[h] == "a" else "gp"
                obj = nc.vector if PPATH[h] == "a" else nc.gpsimd
                p_mk[h] = emit(eng, lambda q, h=h, lo=lo, fin=fin, outP=outP,
                               obj=obj: obj.tensor_tensor(
                                   out=outP, in0=EV[h][:, lo:512], in1=fin,
                                   op=Alu.mult), [cp_mk[h]])

        # ---- packed dot matmuls: d_m (m=1..63) + u^T f_63 -> PS[0][:,0:256]
        wait_p = [mk_w2] + p_mk
        dm = None
        for r in range(16):
            nblk = len(range(r, R - 1, 16))
            dm = emit("pe", lambda q, r=r, nblk=nblk: nc.tensor.matmul(
                PS[0][:, 0:64 * nblk], Wd[r], P_sb[:, r:R - 1:16, :],
                start=(r == 0), stop=False, skip_group_check=True),
                wait_p if r == 0 else [])
        dm = emit("pe", lambda q: nc.tensor.matmul(
            PS[0][:, 192:256], Wu, F_sb[:, (R - 1) * U:R * U],
            start=False, stop=True, skip_group_check=True), [])

        ev_final = emit("act", lambda q: nc.scalar.copy(
            dsb[:], PS[0][:, 0:256]), [dm])
        emit("sp", dma(dout[:], dsb[:]), [ev_final], inc=16)

        # ---------------- emission ----------------
        def run(eng, q):
            hwm = {}
            for fn, waits, inc, sem in PROG[eng]:
                best = {}
                for (weng, wcnt) in waits:
                    if weng == eng:
                        continue
                    best[weng] = max(best.get(weng, 0), wcnt)
                for weng, wcnt in best.items():
                    if hwm.get(weng, 0) < wcnt:
                        q.wait_ge(sems[weng], wcnt)
                        hwm[weng] = wcnt
                instr = fn(q)
                instr.then_inc(sems[sem], inc)

        @block.sync
        def _(sync):
            run("sp", sync)

        @block.tensor
        def _(tensor):
            run("pe", tensor)

        @block.scalar
        def _(scalar):
            run("act", scalar)

        @block.vector
        def _(vector):
            run("dve", vector)

        @block.gpsimd
        def _(gp):
            run("gp", gp)

    return nc


def _blockdiag(M):
    """W[8j+c, 8j'+c] = M[j, j'] — 8 interleaved 16x16 blocks."""
    W = np.zeros((128, 128), np.float32)
    for c in range(NCH):
        W[c::NCH, c::NCH] = M
    return W


def _prep_inputs(emissions, start_np, end_np, trans_np):
    """Host-side prep: weights + per-core quantized g tensors + host e_m."""
    E64 = np.exp(trans_np.astype(np.float64))
    Eb = E64.astype(np.float32).astype(BF16)
    E32 = Eb.astype(np.float32)                 # weight values as on device
    cs = E32.sum(axis=0)                        # (1^T E)_j
    sc = np.exp(start_np.astype(np.float64)).astype(np.float32)
    u_end = np.exp(end_np.astype(np.float64)).astype(np.float32)

    w1 = np.concatenate([
        _blockdiag(E32),
        _blockdiag(cs[:, None] * E32),
    ], axis=1).astype(BF16)                     # [128, 256]

    Wq = _blockdiag(E32.T)
    Wds = []
    for r in range(16):
        Wr = np.zeros((128, 128), np.float32)
        p = np.arange(128)
        Wr[p, 8 * r + p % 8] = 1.0
        Wds.append(Wr)
    Wu = np.zeros((128, 128), np.float32)
    p = np.arange(128)
    Wu[p, 120 + p % 8] = u_end[p // 8]
    w2 = np.concatenate([Wq] + Wds + [Wu], axis=1).astype(BF16)  # [128, 2304]

    # g slabs: [seg, ph, core, chunk, u, state]
    g32 = np.exp(emissions.astype(np.float32) - np.float32(C_SHIFT))
    g7 = g32.reshape(R, L, NCORES, NCH, U, T).copy()
    # fold the segment-0 seed exp(start)/cs into its ph0 slab, rescaled by
    # alpha to keep the fp8 value range; ln(alpha) is subtracted on the host
    alpha = float(cs.mean())
    seed_fac = (sc * alpha / cs).astype(np.float32)
    g7[0, 0] = g7[0, 0] * seed_fac[None, None, None, :]

    def dev_slab(ph, h, dt):
        arr = g7[SEGH * h:SEGH * (h + 1), ph]       # [8, cores, c, u, j]
        dev = arr.transpose(1, 4, 2, 0, 3)          # [cores, j, c, seg, u]
        return np.ascontiguousarray(dev.reshape(NCORES, 128, SEGH * U)).astype(dt)

    g8 = np.empty((NCORES, 128, NR8, 512), FP8)
    for (ph, h), idx in REG8.items():
        g8[:, :, idx, :] = dev_slab(ph, h, FP8)
    g16 = np.empty((NCORES, 128, NR16, 512), BF16)
    for (ph, h), idx in REG16.items():
        g16[:, :, idx, :] = dev_slab(ph, h, BF16)

    # host-side e_m = cs . g_{m,7} with the SAME quantization the device saw
    g7q = np.empty((R, NCORES, NCH, U, T), np.float64)
    for h in range(NHC):
        dt = FP8 if (L - 1, h) in REG8 else BF16
        blk = g7[SEGH * h:SEGH * (h + 1), L - 1]
        g7q[SEGH * h:SEGH * (h + 1)] = blk.astype(dt).astype(np.float64)
    e_host = np.einsum("j,mncuj->mncu", cs.astype(np.float64), g7q[1:])
    e_host = e_host.reshape(R - 1, B)            # [m-1, b] global batch order

    in_maps = [{"g8": g8[core], "g16": g16[core], "w1": w1, "w2": w2}
               for core in range(NCORES)]
    return in_maps, e_host, np.log(alpha)


def _host_score(emissions, tags, masks, start_transitions, end_transitions,
                transitions):
    tags = tags.astype(np.int64)
    b_idx = np.arange(B)
    score = start_transitions[tags[0]] + emissions[0, b_idx, tags[0]]
    trans_sc = transitions[tags[:-1], tags[1:]] * masks[1:]
    emit_sc = np.take_along_axis(
        emissions[1:], tags[1:, :, None], axis=2)[:, :, 0] * masks[1:]
    score = score + trans_sc.sum(0) + emit_sc.sum(0)
    seq_ends = masks.astype(np.int32).sum(0) - 1
    last_tags = tags[seq_ends, b_idx]
    return score + end_transitions[last_tags]


def _host_normalizer(emissions, masks, start_transitions, end_transitions,
                     transitions):
    """Full-precision host fallback (only used when masks aren't all ones)."""
    sc = (start_transitions[None] + emissions[0]).astype(np.float64)
    E64 = np.exp(transitions.astype(np.float64))
    for t in range(1, S):
        m = sc.max(1, keepdims=True)
        nxt = m + np.log(np.exp(sc - m) @ E64) + emissions[t]
        keep = masks[t][:, None] > 0
        sc = np.where(keep, nxt, sc)
    m = sc.max(1, keepdims=True)
    return (
        m[:, 0]
        + np.log(np.exp(sc - m + end_transitions[None]).sum(1))
    ).astype(np.float32)


def kernel(emissions, tags, masks, start_transitions, end_transitions,
           transitions):
    emissions = np.asarray(emissions, np.float32)
    masks_np = np.asarray(masks, np.float32)
    tags_np = np.asarray(tags)
    start_np = np.asarray(start_transitions, np.float32)
    end_np = np.asarray(end_transitions, np.float32)
    trans_np = np.asarray(transitions, np.float32)

    score = _host_score(emissions, tags_np, masks_np, start_np, end_np,
                        trans_np)

    if not np.all(masks_np == 1.0):
        norm = _host_normalizer(emissions, masks_np, start_np, end_np,
                                trans_np)
        return (score - norm).astype(np.float32)

    from concourse.bass_utils import run_bass_kernel_spmd

    if "nc" not in _COMPILED:
        _COMPILED["nc"] = _build_bass()
    nc = _COMPILED["nc"]

    in_maps, e_host, ln_alpha = _prep_inputs(emissions, start_np, end_np,
                                             trans_np)
    res = run_bass_kernel_spmd(nc, in_maps, core_ids=list(range(NCORES)))

    # decode: dout[8r+c, 64*bi+u] = d for slot s=16*bi+r (m=s+1); s=63 = u-term
    norm = np.empty((NCORES, BL), np.float64)
    ln_d = np.empty((NCORES, R - 1, BL), np.float64)
    for core in range(NCORES):
        dd = res.results[core]["dout"].astype(np.float64)
        dd = dd.reshape(16, NCH, 4, U)            # [r, c, bi, u]
        dots = dd.transpose(2, 0, 1, 3).reshape(64, BL)   # slot s=16*bi+r
        ln_d[core] = np.log(dots[:R - 1])
        norm[core] = np.log(dots[R - 1])
    ln_d = ln_d.transpose(1, 0, 2).reshape(R - 1, B)
    norm = norm.reshape(B)
    norm = norm + (ln_d - np.log(e_host)).sum(axis=0) + S * C_SHIFT - ln_alpha
    return (score - norm.astype(np.float64)).astype(np.float32)
